# revision 36
# baseline (speedup 1.0000x reference)
"""Trainium2 Bass kernel: nearest-centroid assignment (vq_codebook).

Computes, for each row of `feats` [N, 512]:
    f = normalize([feats_n, 1])            (L2, with appended ones column)
    pred_n = labelset[argmin_l ||f - c_l||]   with c = initc[labelset]  [128, 513]

Equivalent argmax formulation used on device (monotone transform, per row n):
    argmin_l dist  ==  argmax_l  T[n, l]
    T[n, l] = sum_d feats[n,d] * c[l,d]  +  c[l,512]  -  m_n * w2_l
    m_n  = sqrt(|feats_n|^2 + 1)            (= r_n)
    w2_l = 0.5 * |c_l|^2                    (over all 513 dims)

Sharding: pure data-parallel over rows, N/8 = 32768 rows per NeuronCore.

Per-core dataflow (groups of 512 rows; measured ~313 us/core on TRN2,
all four engines balanced at 81-88% occupancy):
    SWDGE DMA: 512x512 f32 tile, cast to fp16 in-flight (natural layout)
    DVE/ACT : square+accum -> q ; ACT sqrt/shift -> u = r - sqrt(513)
    PE      : transpose u-columns into a [1, 512] fp16 row
    PE      : 16x transpose 128x128 fp16 feats blocks -> fT
    DVE/ACT : copy fT PSUM->SBUF (fp16, 2x DVE mode)
    PE      : G[l, n] = sum_k cT16_k.T @ fT_k  (+ fp16 rank-1 terms
              cdv_hi*ones + cdv_lo*ones + (-w2c)*u_row), fp32 PSUM accum
    ACT     : copy G -> SBUF;  PE: transpose G -> T[n, l] in exact fp32
    DVE     : max / max_index over l -> argmax index per row

Precision design: centering w2 (per-row shift) and m (constant shift, folded
into the per-l constant cdv, split hi/lo so fp16 holds it exactly) keeps every
fp16-rounded term at least 20x below the top-2 score gaps; scores themselves
(large dynamic range) stay fp32 through the final transpose and argmax.
Measured 1/262144 label mismatches vs the fp32 reference.

Notes on this walrus build:
  - one sync wait per engine instruction: constants ship in single-DMA packs
    (one semaphore lane each) + PE/ACT warmup ops observe them; any residual
    multi-wait instruction is split via same-engine NoOps (_split_multiwait).
  - fp16 and fp32r matmuls must not share a PSUM accumulation group (silent
    corruption on hardware); every matmul in the group is fp16.
"""

import os
import sys

import numpy as np

for _p in ("/opt/trn_rl_repo",):
    if _p not in sys.path and os.path.isdir(_p):
        sys.path.insert(0, _p)

import concourse.bass as bass
import concourse.mybir as mybir
import concourse.tile as tile
from concourse.bass_utils import run_bass_kernel_spmd

N, D, K = 262144, 512, 128
N_CORES = 8
ROWS_PER_CORE = N // N_CORES  # 32768
GROUP = 512  # rows per group (4 subtiles of 128)

F32 = mybir.dt.float32
F32R = mybir.dt.float32r
F16 = mybir.dt.float16
U32 = mybir.dt.uint32
AF = mybir.ActivationFunctionType

# cpack layout (columns of one [128, 1536] f32 constant block, single DMA)
IDENT_OFF = 0      # [128, 128] identity
CT_OFF = 128       # [128, 512] cT: cT[p, 128k+l] = c[l, 128k+p]
CD_OFF = 640       # row 0, [1, 128]  hi part of (cd - MBAR*w2c)
W2_OFF = 768       # row 0, [1, 128]  -(w2 - mean(w2))
CD2_OFF = 1424     # row 0, [1, 128]  lo part of (cd - MBAR*w2c)
ONES_OFF = 896     # row 0, [1, 512]  ones
BIAS1_OFF = 1408   # [128, 1] column of 1.0 (sqrt bias)
BIAS0_OFF = 1409   # [128, 1] column of 0.0 (square bias; avoids const-AP dep)
ID16_OFF = 1600    # [128, 64] f32 = [128, 128] fp16 identity (bit view)
CT16_OFF = 1664    # [128, 256] f32 = [128, 512] fp16 cT (bit view)
W216_OFF = 1920    # row 0, [1, 64] f32 = [1, 128] fp16 -w2c (bit view)
CPACK_W = 2048

# native fp16 constant pack (separate DRAM param, fp16 dtype end to end)
P16_IDENT = 0      # [128, 128] identity
P16_CT = 128       # [128, 512] cT
P16_CDVH = 640     # row 0, [1, 128] cdv hi (bf16-exact)
P16_CDVL = 768     # row 0, [1, 128] cdv lo
P16_W2 = 896       # row 0, [1, 128] -w2c
P16_ONES = 1024    # row 0, [1, 512] ones
P16_W = 1536

USE_F32R = True
MBAR = float(np.sqrt(513.0))  # fixed shift for m; any constant is argmax-invariant


def _mm(ap):
    if USE_F32R and ap.dtype == F32:
        return ap.bitcast(F32R)
    return ap


def _split_multiwait(nc):
    """Walrus (this build) allows one sync wait per engine instruction.

    Tile occasionally emits 2+ (data dep + buffer-slot release on another
    semaphore). Splitting is semantics-preserving: a same-engine NoOp placed
    immediately before the instruction carries the surplus waits; the engine
    executes in order, so all waits are still enforced before the instruction
    runs.
    """
    import bass_rust

    for fn in nc.m.functions:
        for blk in fn.blocks:
            out = []
            changed = False
            for inst in blk.instructions:
                si = getattr(inst, "sync_info", None)
                waits = list(si.on_wait) if si is not None else []
                if len(waits) > 1:
                    for w in waits[:-1]:
                        nop = mybir.InstNoOp(
                            name=nc.get_next_instruction_name(), ins=[], outs=[]
                        )
                        nop.engine = inst.engine
                        nop.sync_info = bass_rust.SyncInfo(
                            on_wait=[w], on_update=[]
                        )
                        out.append(nop)
                    inst.sync_info = bass_rust.SyncInfo(
                        on_wait=waits[-1:], on_update=list(si.on_update)
                    )
                    changed = True
                out.append(inst)
            if changed:
                blk.instructions = out


def build_core_program(rows=ROWS_PER_CORE, split_waits=True):
    """Bass program for one NeuronCore processing `rows` rows of feats."""
    assert rows % GROUP == 0
    ngroups = rows // GROUP
    ntiles = rows // 128

    out_chunk = min(16, ngroups)
    nc = bass.Bass()
    feats_d = nc.declare_dram_parameter("feats", [rows, D], F32R if USE_F32R else F32, isOutput=False)
    cpack_d = nc.declare_dram_parameter("cpack", [128, CPACK_W], F32R if USE_F32R else F32, isOutput=False)
    cpk16_d = nc.declare_dram_parameter("cpk16", [128, P16_W], F16, isOutput=False)
    idx_d = nc.declare_dram_parameter("idx8", [128, ntiles, 8], U32, isOutput=True)

    with tile.TileContext(nc) as tc:
        with (
            tc.tile_pool(name="const", bufs=1) as constp,
            tc.tile_pool(name="fin", bufs=6) as finp,
            tc.tile_pool(name="sq", bufs=2) as sqp,
            tc.tile_pool(name="ftsb", bufs=4) as ftsbp,
            tc.tile_pool(name="gs", bufs=2) as gsp,
            tc.tile_pool(name="st", bufs=3) as stp,
            tc.tile_pool(name="small", bufs=4) as smallp,
            tc.tile_pool(name="outp", bufs=1) as outp,
            tc.tile_pool(name="ftp", bufs=2, space="PSUM") as ftpp,
            tc.tile_pool(name="gp", bufs=2, space="PSUM") as gpp,
            tc.tile_pool(name="gtp", bufs=1, space="PSUM") as gtpp,
            tc.tile_pool(name="mrp", bufs=1, space="PSUM") as mrpp,
        ):
            cpack = constp.tile([128, CPACK_W], F32R if USE_F32R else F32)
            nc.sync.dma_start(cpack[:], cpack_d[:])
            ident = cpack[:, IDENT_OFF : IDENT_OFF + 128]
            cT = cpack[:, CT_OFF : CT_OFF + 512]
            cd_row = cpack[0:1, CD_OFF : CD_OFF + 128]
            cd2_row = cpack[0:1, CD2_OFF : CD2_OFF + 128]
            w2neg_row = cpack[0:1, W2_OFF : W2_OFF + 128]
            ones = cpack[0:1, ONES_OFF : ONES_OFF + 512]
            cpk16 = constp.tile([128, P16_W], F16)
            nc.sync.dma_start(cpk16[:], cpk16_d[:])
            ident16 = cpk16[:, P16_IDENT : P16_IDENT + 128]
            cT16 = cpk16[:, P16_CT : P16_CT + 512]
            cdvh16_row = cpk16[0:1, P16_CDVH : P16_CDVH + 128]
            cdvl16_row = cpk16[0:1, P16_CDVL : P16_CDVL + 128]
            w2neg16_row = cpk16[0:1, P16_W2 : P16_W2 + 128]
            ones16 = cpk16[0:1, P16_ONES : P16_ONES + 512]
            one_bias = cpack[:, BIAS1_OFF : BIAS1_OFF + 1]
            zero_bias = cpack[:, BIAS0_OFF : BIAS0_OFF + 1]
            idxacc = outp.tile([128, ntiles, 8], U32)

            # warmup: make the PE observe the const DMA lane with a single-wait
            # instruction, so every later matmul carries at most one new wait.
            warm_ps = mrpp.tile([128, 128], F16, tag="mrow_ps")
            nc.tensor.transpose(warm_ps[:], ident16, ident16)
            warm2_ps = mrpp.tile([128, 128], F32, tag="mrow_ps")
            ident32w = ident.bitcast(F32) if USE_F32R else ident
            nc.tensor.transpose(warm2_ps[:], ident32w, ident32w)
            act_warm = smallp.tile([1, 1], F32, tag="act_warm")
            nc.scalar.copy(act_warm[:], cpack[0:1, 0:1])

            for g in range(ngroups):
                Fg = finp.tile([128, 4, 512], F16)
                srcap = feats_d[g * GROUP : (g + 1) * GROUP, :].rearrange(
                    "(j p) d -> p j d", p=128
                )
                nc.gpsimd.dma_start(Fg[:], srcap)

                # q_j = |feats_row|^2 per partition; m = sqrt(q+1)
                Q = smallp.tile([128, 4], F32, tag="Q")
                R = smallp.tile([128, 4], F32, tag="R")
                Mcol = smallp.tile([128, 4], F16, tag="Mcol")
                for j in range(4):
                    sq = sqp.tile([128, 512], F32)
                    if j % 2 == 0:
                        nc.vector.scalar_tensor_tensor(
                            sq[:], Fg[:, j, :], 1.0, Fg[:, j, :],
                            mybir.AluOpType.mult, mybir.AluOpType.mult,
                            accum_out=Q[:, j : j + 1],
                        )
                    else:
                        nc.scalar.activation(
                            sq[:], Fg[:, j, :], AF.Square,
                            bias=zero_bias, accum_out=Q[:, j : j + 1],
                        )
                nc.scalar.activation(R[:], Q[:], AF.Sqrt, bias=one_bias, scale=1.0)
                # u = r - MBAR keeps the m*w2 rank-1 term small enough that
                # fp32r rounding stays far below the score gaps
                nc.scalar.activation(Mcol[:], R[:], AF.Copy, bias=-MBAR, scale=1.0)
                # m columns -> one [1, 512] row via PE transpose
                mrow_ps = mrpp.tile([1, 512], F16, tag="mrow_ps")
                for j in range(4):
                    nc.tensor.transpose(
                        mrow_ps[:, j * 128 : (j + 1) * 128],
                        Mcol[:, j : j + 1],
                        ident16,
                    )
                mrow = smallp.tile([1, 512], F16, tag="mrow")
                nc.scalar.copy(mrow[:], mrow_ps[:])

                # G[l, n] accumulation over 4 contraction chunks + 2 rank-1 terms
                G_ps = gpp.tile([128, 512], F32)
                for kk in range(2):
                    ft_ps = ftpp.tile([128, 2, 512], F16)
                    for k2 in range(2):
                        k = 2 * kk + k2
                        for j in range(4):
                            nc.tensor.transpose(
                                ft_ps[:, k2, j * 128 : (j + 1) * 128],
                                Fg[:, j, k * 128 : (k + 1) * 128],
                                ident16,
                            )
                    ft = ftsbp.tile([128, 2, 512], F16)
                    if kk == 0:
                        nc.vector.tensor_copy(ft[:], ft_ps[:])
                    else:
                        nc.scalar.copy(ft[:], ft_ps[:])
                    for k2 in range(2):
                        k = 2 * kk + k2
                        nc.tensor.matmul(
                            G_ps[:],
                            cT16[:, k * 128 : (k + 1) * 128],
                            ft[:, k2, :],
                            start=(k == 0),
                            stop=False,
                        )
                nc.tensor.matmul(
                    G_ps[:], cdvh16_row, ones16, start=False, stop=False
                )
                nc.tensor.matmul(
                    G_ps[:], cdvl16_row, ones16, start=False, stop=False
                )
                nc.tensor.matmul(
                    G_ps[:], w2neg16_row, mrow[:], start=False, stop=True
                )

                # transpose G -> T[n, l], then per-row argmax over l
                Gs = gsp.tile([128, 512], F32)
                nc.scalar.copy(Gs[:], G_ps[:])
                gt_ps = gtpp.tile([128, 512], F32)
                ident32 = ident.bitcast(F32) if USE_F32R else ident
                for j in range(4):
                    nc.tensor.transpose(
                        gt_ps[:, j * 128 : (j + 1) * 128],
                        Gs[:, j * 128 : (j + 1) * 128],
                        ident32,
                    )
                st = stp.tile([128, 512], F32)
                nc.vector.tensor_copy(st[:], gt_ps[:])
                for j in range(4):
                    mx = smallp.tile([128, 8], F32, tag="mx")
                    nc.vector.max(mx[:], st[:, j * 128 : (j + 1) * 128])
                    nc.vector.max_index(
                        idxacc[:, g * 4 + j, :], mx[:], st[:, j * 128 : (j + 1) * 128]
                    )
                # stream results out in chunks so the final flush overlaps
                # compute instead of running after the drain
                if (g + 1) % out_chunk == 0:
                    t0, t1 = (g + 1 - out_chunk) * 4, (g + 1) * 4
                    nc.sync.dma_start(idx_d[:, t0:t1, :], idxacc[:, t0:t1, :])

            if ngroups % out_chunk:
                t0 = (ngroups - ngroups % out_chunk) * 4
                nc.sync.dma_start(idx_d[:, t0:, :], idxacc[:, t0:, :])
    if split_waits:
        _split_multiwait(nc)
    return nc


def make_const_inputs(initc, labelset):
    c = np.asarray(initc, dtype=np.float32)[np.asarray(labelset).astype(np.int64)]
    assert c.shape == (K, D + 1)
    cpack = np.zeros((128, CPACK_W), np.float32)
    cpack[:, IDENT_OFF : IDENT_OFF + 128] = np.eye(128, dtype=np.float32)
    for k in range(4):
        cpack[:, CT_OFF + k * 128 : CT_OFF + (k + 1) * 128] = (
            c[:, k * 128 : (k + 1) * 128].T
        )
    w2 = 0.5 * np.sum(c.astype(np.float64) ** 2, axis=1)
    # Centering w2 and m shifts scores by per-row constants (argmax invariant)
    # while shrinking the fp32r-rounded rank-1 magnitudes by >10x. The
    # leftover per-l constant cdv = cd - MBAR*w2c is large, so it ships as a
    # bf16-exact hi part (representable in fp32r) plus a small lo remainder.
    w2c = w2 - w2.mean()
    cdv = c[:, D].astype(np.float64) - MBAR * w2c
    cdv_hi = cdv.astype(np.float32).astype(np.dtype("float32"))
    cdv_hi = (cdv_hi.view(np.uint32) & np.uint32(0xFFFF0000)).view(np.float32)
    cdv_lo = (cdv - cdv_hi.astype(np.float64)).astype(np.float32)
    cpack[0, CD_OFF : CD_OFF + 128] = cdv_hi
    cpack[0, CD2_OFF : CD2_OFF + 128] = cdv_lo
    cpack[0, W2_OFF : W2_OFF + 128] = -w2c.astype(np.float32)
    cpack[0, ONES_OFF : ONES_OFF + 512] = 1.0
    cpack[:, BIAS1_OFF] = 1.0
    def put16(col, rows, arr16):
        bits = arr16.astype(np.float16).view(np.uint16).astype(np.uint32)
        packed = (bits[..., 0::2] | (bits[..., 1::2] << np.uint32(16))).view(np.float32)
        cpack[rows, col : col + packed.shape[-1]] = packed
    cpk16 = np.zeros((128, P16_W), np.float16)
    cpk16[:, P16_IDENT : P16_IDENT + 128] = np.eye(128, dtype=np.float16)
    for k in range(4):
        cpk16[:, P16_CT + k * 128 : P16_CT + (k + 1) * 128] = (
            c[:, k * 128 : (k + 1) * 128].T.astype(np.float16)
        )
    cdv_hi16 = cdv_hi.astype(np.float16)
    assert np.array_equal(cdv_hi16.astype(np.float32), cdv_hi), "cdv_hi not fp16-exact"
    cpk16[0, P16_CDVH : P16_CDVH + 128] = cdv_hi16
    cpk16[0, P16_CDVL : P16_CDVL + 128] = cdv_lo.astype(np.float16)
    cpk16[0, P16_W2 : P16_W2 + 128] = (-w2c).astype(np.float16)
    cpk16[0, P16_ONES : P16_ONES + 512] = 1.0
    return {"cpack": cpack, "cpk16": cpk16}


def kernel(feats, initc, labelset):
    feats = np.asarray(feats, dtype=np.float32)
    labelset_np = np.asarray(labelset)
    consts = make_const_inputs(initc, labelset)

    nc = build_core_program(ROWS_PER_CORE)
    in_maps = []
    for core in range(N_CORES):
        shard = feats[core * ROWS_PER_CORE : (core + 1) * ROWS_PER_CORE]
        in_maps.append({"feats": np.ascontiguousarray(shard), **consts})
    res = run_bass_kernel_spmd(nc, in_maps, list(range(N_CORES)))

    preds = []
    for core in range(N_CORES):
        idx8 = np.asarray(res.results[core]["idx8"])  # [128, ntiles, 8]
        idx = idx8[:, :, 0].T.reshape(-1).astype(np.int64)  # row 128*t+p
        preds.append(idx)
    idx_all = np.concatenate(preds)
    return labelset_np[idx_all]


# revision 37
# speedup vs baseline: 1.1888x; 1.1888x over previous
"""Trainium2 Bass kernel: nearest-centroid assignment (vq_codebook).

Computes, for each row of `feats` [N, 512]:
    f = normalize([feats_n, 1])            (L2, with appended ones column)
    pred_n = labelset[argmin_l ||f - c_l||]   with c = initc[labelset]  [128, 513]

Equivalent argmax formulation used on device (monotone transform, per row n):
    argmin_l dist  ==  argmax_l  T[n, l]
    T[n, l] = sum_d feats[n,d] * c[l,d]  +  c[l,512]  -  m_n * w2_l
    m_n  = sqrt(|feats_n|^2 + 1)            (= r_n)
    w2_l = 0.5 * |c_l|^2                    (over all 513 dims)

Sharding: pure data-parallel over rows, N/8 = 32768 rows per NeuronCore.

Per-core dataflow (groups of 512 rows; measured ~313 us/core on TRN2,
all four engines balanced at 81-88% occupancy):
    SWDGE DMA: 512x512 f32 tile, cast to fp16 in-flight (natural layout)
    DVE/ACT : square+accum -> q ; ACT sqrt/shift -> u = r - sqrt(513)
    PE      : transpose u-columns into a [1, 512] fp16 row
    PE      : 16x transpose 128x128 fp16 feats blocks -> fT
    DVE/ACT : copy fT PSUM->SBUF (fp16, 2x DVE mode)
    PE      : G[l, n] = sum_k cT16_k.T @ fT_k  (+ fp16 rank-1 terms
              cdv_hi*ones + cdv_lo*ones + (-w2c)*u_row), fp32 PSUM accum
    ACT     : copy G -> SBUF;  PE: transpose G -> T[n, l] in exact fp32
    DVE     : max / max_index over l -> argmax index per row

Precision design: centering w2 (per-row shift) and m (constant shift, folded
into the per-l constant cdv, split hi/lo so fp16 holds it exactly) keeps every
fp16-rounded term at least 20x below the top-2 score gaps; scores themselves
(large dynamic range) stay fp32 through the final transpose and argmax.
Measured 1/262144 label mismatches vs the fp32 reference.

Notes on this walrus build:
  - one sync wait per engine instruction: constants ship in single-DMA packs
    (one semaphore lane each) + PE/ACT warmup ops observe them; any residual
    multi-wait instruction is split via same-engine NoOps (_split_multiwait).
  - fp16 and fp32r matmuls must not share a PSUM accumulation group (silent
    corruption on hardware); every matmul in the group is fp16.
"""

import os
import sys

import numpy as np

for _p in ("/opt/trn_rl_repo",):
    if _p not in sys.path and os.path.isdir(_p):
        sys.path.insert(0, _p)

import concourse.bass as bass
import concourse.mybir as mybir
import concourse.tile as tile
from concourse.bass_utils import run_bass_kernel_spmd

N, D, K = 262144, 512, 128
N_CORES = 8
ROWS_PER_CORE = N // N_CORES  # 32768
GROUP = 512  # rows per group (4 subtiles of 128)

F32 = mybir.dt.float32
F32R = mybir.dt.float32r
F16 = mybir.dt.float16
U32 = mybir.dt.uint32
AF = mybir.ActivationFunctionType

# cpack layout (columns of one [128, 1536] f32 constant block, single DMA)
IDENT_OFF = 0      # [128, 128] identity
CT_OFF = 128       # [128, 512] cT: cT[p, 128k+l] = c[l, 128k+p]
CD_OFF = 640       # row 0, [1, 128]  hi part of (cd - MBAR*w2c)
W2_OFF = 768       # row 0, [1, 128]  -(w2 - mean(w2))
CD2_OFF = 1424     # row 0, [1, 128]  lo part of (cd - MBAR*w2c)
ONES_OFF = 896     # row 0, [1, 512]  ones
BIAS1_OFF = 1408   # [128, 1] column of 1.0 (sqrt bias)
BIAS0_OFF = 1409   # [128, 1] column of 0.0 (square bias; avoids const-AP dep)
ID16_OFF = 1600    # [128, 64] f32 = [128, 128] fp16 identity (bit view)
CT16_OFF = 1664    # [128, 256] f32 = [128, 512] fp16 cT (bit view)
W216_OFF = 1920    # row 0, [1, 64] f32 = [1, 128] fp16 -w2c (bit view)
CPACK_W = 2048

# native fp16 constant pack (separate DRAM param, fp16 dtype end to end)
P16_IDENT = 0      # [128, 128] identity
P16_CT = 128       # [128, 512] cT
P16_CDVH = 640     # row 0, [1, 128] cdv hi (bf16-exact)
P16_CDVL = 768     # row 0, [1, 128] cdv lo
P16_W2 = 896       # row 0, [1, 128] -w2c
P16_ONES = 1024    # row 0, [1, 512] ones
P16_W = 1536

USE_F32R = True
MBAR = float(np.sqrt(513.0))  # fixed shift for m; any constant is argmax-invariant


def _mm(ap):
    if USE_F32R and ap.dtype == F32:
        return ap.bitcast(F32R)
    return ap


def _split_multiwait(nc):
    """Walrus (this build) allows one sync wait per engine instruction.

    Tile occasionally emits 2+ (data dep + buffer-slot release on another
    semaphore). Splitting is semantics-preserving: a same-engine NoOp placed
    immediately before the instruction carries the surplus waits; the engine
    executes in order, so all waits are still enforced before the instruction
    runs.
    """
    import bass_rust

    for fn in nc.m.functions:
        for blk in fn.blocks:
            out = []
            changed = False
            for inst in blk.instructions:
                si = getattr(inst, "sync_info", None)
                waits = list(si.on_wait) if si is not None else []
                if len(waits) > 1:
                    for w in waits[:-1]:
                        nop = mybir.InstNoOp(
                            name=nc.get_next_instruction_name(), ins=[], outs=[]
                        )
                        nop.engine = inst.engine
                        nop.sync_info = bass_rust.SyncInfo(
                            on_wait=[w], on_update=[]
                        )
                        out.append(nop)
                    inst.sync_info = bass_rust.SyncInfo(
                        on_wait=waits[-1:], on_update=list(si.on_update)
                    )
                    changed = True
                out.append(inst)
            if changed:
                blk.instructions = out


def build_core_program(rows=ROWS_PER_CORE, split_waits=True):
    """Bass program for one NeuronCore processing `rows` rows of feats."""
    assert rows % GROUP == 0
    ngroups = rows // GROUP
    ntiles = rows // 128

    nc = bass.Bass()
    feats_d = nc.declare_dram_parameter("feats", [rows, D], F32R if USE_F32R else F32, isOutput=False)
    cpack_d = nc.declare_dram_parameter("cpack", [128, CPACK_W], F32R if USE_F32R else F32, isOutput=False)
    cpk16_d = nc.declare_dram_parameter("cpk16", [128, P16_W], F16, isOutput=False)
    idx_d = nc.declare_dram_parameter("idx8", [128, ntiles, 8], U32, isOutput=True)

    with tile.TileContext(nc) as tc:
        with (
            tc.tile_pool(name="const", bufs=1) as constp,
            tc.tile_pool(name="fin", bufs=6) as finp,
            tc.tile_pool(name="sq", bufs=2) as sqp,
            tc.tile_pool(name="ftsb", bufs=4) as ftsbp,
            tc.tile_pool(name="gs", bufs=2) as gsp,
            tc.tile_pool(name="st", bufs=3) as stp,
            tc.tile_pool(name="small", bufs=4) as smallp,
            tc.tile_pool(name="outp", bufs=1) as outp,
            tc.tile_pool(name="ftp", bufs=2, space="PSUM") as ftpp,
            tc.tile_pool(name="gp", bufs=2, space="PSUM") as gpp,
            tc.tile_pool(name="gtp", bufs=1, space="PSUM") as gtpp,
            tc.tile_pool(name="mrp", bufs=1, space="PSUM") as mrpp,
        ):
            cpack = constp.tile([128, CPACK_W], F32R if USE_F32R else F32)
            nc.sync.dma_start(cpack[:], cpack_d[:])
            ident = cpack[:, IDENT_OFF : IDENT_OFF + 128]
            cT = cpack[:, CT_OFF : CT_OFF + 512]
            cd_row = cpack[0:1, CD_OFF : CD_OFF + 128]
            cd2_row = cpack[0:1, CD2_OFF : CD2_OFF + 128]
            w2neg_row = cpack[0:1, W2_OFF : W2_OFF + 128]
            ones = cpack[0:1, ONES_OFF : ONES_OFF + 512]
            cpk16 = constp.tile([128, P16_W], F16)
            nc.sync.dma_start(cpk16[:], cpk16_d[:])
            ident16 = cpk16[:, P16_IDENT : P16_IDENT + 128]
            cT16 = cpk16[:, P16_CT : P16_CT + 512]
            cdvh16_row = cpk16[0:1, P16_CDVH : P16_CDVH + 128]
            cdvl16_row = cpk16[0:1, P16_CDVL : P16_CDVL + 128]
            w2neg16_row = cpk16[0:1, P16_W2 : P16_W2 + 128]
            ones16 = cpk16[0:1, P16_ONES : P16_ONES + 512]
            one_bias = cpack[:, BIAS1_OFF : BIAS1_OFF + 1]
            zero_bias = cpack[:, BIAS0_OFF : BIAS0_OFF + 1]
            idxacc = outp.tile([128, ntiles, 8], U32)

            # warmup: make the PE observe the const DMA lane with a single-wait
            # instruction, so every later matmul carries at most one new wait.
            warm_ps = mrpp.tile([128, 128], F16, tag="mrow_ps")
            nc.tensor.transpose(warm_ps[:], ident16, ident16)
            warm2_ps = mrpp.tile([128, 128], F32, tag="mrow_ps")
            ident32w = ident.bitcast(F32) if USE_F32R else ident
            nc.tensor.transpose(warm2_ps[:], ident32w, ident32w)
            act_warm = smallp.tile([1, 1], F32, tag="act_warm")
            nc.scalar.copy(act_warm[:], cpack[0:1, 0:1])

            for g in range(ngroups):
                Fg = finp.tile([128, 4, 512], F16)
                srcap = feats_d[g * GROUP : (g + 1) * GROUP, :].rearrange(
                    "(j p) d -> p j d", p=128
                )
                nc.gpsimd.dma_start(Fg[:], srcap)

                # q_j = |feats_row|^2 per partition; m = sqrt(q+1)
                Q = smallp.tile([128, 4], F32, tag="Q")
                R = smallp.tile([128, 4], F32, tag="R")
                Mcol = smallp.tile([128, 4], F16, tag="Mcol")
                for j in range(4):
                    sq = sqp.tile([128, 512], F32)
                    if j % 2 == 0:
                        nc.vector.scalar_tensor_tensor(
                            sq[:], Fg[:, j, :], 1.0, Fg[:, j, :],
                            mybir.AluOpType.mult, mybir.AluOpType.mult,
                            accum_out=Q[:, j : j + 1],
                        )
                    else:
                        nc.scalar.activation(
                            sq[:], Fg[:, j, :], AF.Square,
                            bias=zero_bias, accum_out=Q[:, j : j + 1],
                        )
                nc.scalar.activation(R[:], Q[:], AF.Sqrt, bias=one_bias, scale=1.0)
                # u = r - MBAR keeps the m*w2 rank-1 term small enough that
                # fp32r rounding stays far below the score gaps
                nc.scalar.activation(Mcol[:], R[:], AF.Copy, bias=-MBAR, scale=1.0)
                # m columns -> one [1, 512] row via PE transpose
                mrow_ps = mrpp.tile([1, 512], F16, tag="mrow_ps")
                for j in range(4):
                    nc.tensor.transpose(
                        mrow_ps[:, j * 128 : (j + 1) * 128],
                        Mcol[:, j : j + 1],
                        ident16,
                    )
                mrow = smallp.tile([1, 512], F16, tag="mrow")
                nc.scalar.copy(mrow[:], mrow_ps[:])

                # G[l, n] accumulation over 4 contraction chunks + 2 rank-1 terms
                G_ps = gpp.tile([128, 512], F32)
                for kk in range(2):
                    ft_ps = ftpp.tile([128, 2, 512], F16)
                    for k2 in range(2):
                        k = 2 * kk + k2
                        for j in range(4):
                            nc.tensor.transpose(
                                ft_ps[:, k2, j * 128 : (j + 1) * 128],
                                Fg[:, j, k * 128 : (k + 1) * 128],
                                ident16,
                            )
                    ft = ftsbp.tile([128, 2, 512], F16)
                    if kk == 0:
                        nc.vector.tensor_copy(ft[:], ft_ps[:])
                    else:
                        nc.scalar.copy(ft[:], ft_ps[:])
                    for k2 in range(2):
                        k = 2 * kk + k2
                        nc.tensor.matmul(
                            G_ps[:],
                            cT16[:, k * 128 : (k + 1) * 128],
                            ft[:, k2, :],
                            start=(k == 0),
                            stop=False,
                        )
                nc.tensor.matmul(
                    G_ps[:], cdvh16_row, ones16, start=False, stop=False
                )
                nc.tensor.matmul(
                    G_ps[:], cdvl16_row, ones16, start=False, stop=False
                )
                nc.tensor.matmul(
                    G_ps[:], w2neg16_row, mrow[:], start=False, stop=True
                )

                # transpose G -> T[n, l], then per-row argmax over l
                Gs = gsp.tile([128, 512], F32)
                nc.scalar.copy(Gs[:], G_ps[:])
                gt_ps = gtpp.tile([128, 512], F32)
                ident32 = ident.bitcast(F32) if USE_F32R else ident
                for j in range(4):
                    nc.tensor.transpose(
                        gt_ps[:, j * 128 : (j + 1) * 128],
                        Gs[:, j * 128 : (j + 1) * 128],
                        ident32,
                    )
                st = stp.tile([128, 512], F32)
                nc.vector.tensor_copy(st[:], gt_ps[:])
                for j in range(4):
                    mx = smallp.tile([128, 8], F32, tag="mx")
                    nc.vector.max(mx[:], st[:, j * 128 : (j + 1) * 128])
                    nc.vector.max_index(
                        idxacc[:, g * 4 + j, :], mx[:], st[:, j * 128 : (j + 1) * 128]
                    )

            nc.sync.dma_start(idx_d[:], idxacc[:])
    if split_waits:
        _split_multiwait(nc)
    return nc


def make_const_inputs(initc, labelset):
    c = np.asarray(initc, dtype=np.float32)[np.asarray(labelset).astype(np.int64)]
    assert c.shape == (K, D + 1)
    cpack = np.zeros((128, CPACK_W), np.float32)
    cpack[:, IDENT_OFF : IDENT_OFF + 128] = np.eye(128, dtype=np.float32)
    for k in range(4):
        cpack[:, CT_OFF + k * 128 : CT_OFF + (k + 1) * 128] = (
            c[:, k * 128 : (k + 1) * 128].T
        )
    w2 = 0.5 * np.sum(c.astype(np.float64) ** 2, axis=1)
    # Centering w2 and m shifts scores by per-row constants (argmax invariant)
    # while shrinking the fp32r-rounded rank-1 magnitudes by >10x. The
    # leftover per-l constant cdv = cd - MBAR*w2c is large, so it ships as a
    # bf16-exact hi part (representable in fp32r) plus a small lo remainder.
    w2c = w2 - w2.mean()
    cdv = c[:, D].astype(np.float64) - MBAR * w2c
    cdv_hi = cdv.astype(np.float32).astype(np.dtype("float32"))
    cdv_hi = (cdv_hi.view(np.uint32) & np.uint32(0xFFFF0000)).view(np.float32)
    cdv_lo = (cdv - cdv_hi.astype(np.float64)).astype(np.float32)
    cpack[0, CD_OFF : CD_OFF + 128] = cdv_hi
    cpack[0, CD2_OFF : CD2_OFF + 128] = cdv_lo
    cpack[0, W2_OFF : W2_OFF + 128] = -w2c.astype(np.float32)
    cpack[0, ONES_OFF : ONES_OFF + 512] = 1.0
    cpack[:, BIAS1_OFF] = 1.0
    def put16(col, rows, arr16):
        bits = arr16.astype(np.float16).view(np.uint16).astype(np.uint32)
        packed = (bits[..., 0::2] | (bits[..., 1::2] << np.uint32(16))).view(np.float32)
        cpack[rows, col : col + packed.shape[-1]] = packed
    cpk16 = np.zeros((128, P16_W), np.float16)
    cpk16[:, P16_IDENT : P16_IDENT + 128] = np.eye(128, dtype=np.float16)
    for k in range(4):
        cpk16[:, P16_CT + k * 128 : P16_CT + (k + 1) * 128] = (
            c[:, k * 128 : (k + 1) * 128].T.astype(np.float16)
        )
    cdv_hi16 = cdv_hi.astype(np.float16)
    assert np.array_equal(cdv_hi16.astype(np.float32), cdv_hi), "cdv_hi not fp16-exact"
    cpk16[0, P16_CDVH : P16_CDVH + 128] = cdv_hi16
    cpk16[0, P16_CDVL : P16_CDVL + 128] = cdv_lo.astype(np.float16)
    cpk16[0, P16_W2 : P16_W2 + 128] = (-w2c).astype(np.float16)
    cpk16[0, P16_ONES : P16_ONES + 512] = 1.0
    return {"cpack": cpack, "cpk16": cpk16}


def kernel(feats, initc, labelset):
    feats = np.asarray(feats, dtype=np.float32)
    labelset_np = np.asarray(labelset)
    consts = make_const_inputs(initc, labelset)

    nc = build_core_program(ROWS_PER_CORE)
    in_maps = []
    for core in range(N_CORES):
        shard = feats[core * ROWS_PER_CORE : (core + 1) * ROWS_PER_CORE]
        in_maps.append({"feats": np.ascontiguousarray(shard), **consts})
    res = run_bass_kernel_spmd(nc, in_maps, list(range(N_CORES)))

    preds = []
    for core in range(N_CORES):
        idx8 = np.asarray(res.results[core]["idx8"])  # [128, ntiles, 8]
        idx = idx8[:, :, 0].T.reshape(-1).astype(np.int64)  # row 128*t+p
        preds.append(idx)
    idx_all = np.concatenate(preds)
    return labelset_np[idx_all]
